# revision 17
# baseline (speedup 1.0000x reference)
"""Trainium2 Bass kernel: causal multi-head attention with LoRA (B=2, T=2048,
C=1024, 16 heads, r=16), SPMD across 8 NeuronCores.

Sharding: core = (batch, head-group-of-4). QKV + attention are fully local per
core; the output projection is a partial sum over each core's 256 y-features,
reduced on host.

Host-side exact folds (no HW cost):
  - LoRA:  W_eff = W + (1/r) * B @ A        (both attn and proj)
  - k-bias: drops out of softmax (constant shift per query)
  - v-bias: y = sum(p*(v+bv)) = sum(p*v) + bv  ->  folded into proj bias
  - q-bias: adds (bq . k_t) to every score column; k is linear in x, so it is
    one extra projection feature (wstar = W_k_eff^T bq); applied post-exp as a
    per-partition multiply only when any q-bias is nonzero (variant flag).

Device schedule (single NeuronCore, emission order == per-engine order):
  qk f0/f2 proj -> v tiles 0-3 -> attention units (h-major), software
  pipelined: unit U's score matmuls interleave with unit U-1's AV matmuls so
  the PE never waits on the exp chain; remaining v tiles and qk f1/f3 are
  dribbled into the attention stream as PE filler to keep the tensor engine
  p-state at max clock. Scores are written as [128,2,512] 2-bank PSUM tiles so
  one Exp instruction covers two k-tiles (halves Act instruction count).
  Output projection drains at the tail, alternating Act/DVE PSUM reads.
"""
import os
import sys

sys.path.insert(0, "/opt/trn_rl_repo")

import numpy as np

import concourse.bass as bass  # noqa: F401
import concourse.bacc as bacc
import concourse.tile as tile
import concourse.mybir as mybir
from concourse.bass_utils import run_bass_kernel_spmd

B, T, C = 2, 2048, 1024
H, HD = 16, 64
R = 16
LORA_SCALE = 1.0 / R
N_CORES = 8
GPB = N_CORES // B          # core groups per batch = 4
HPC = H // GPB              # heads per core = 4
CI = HPC * HD               # per-core y features = 256
P = 128
T5 = T // 512               # 4  (512-wide t tiles)
T1 = T // P                 # 16 (128-wide t tiles)
CT = C // P                 # 8  (128-wide c tiles)
FQK = 4                     # 128-wide qk feature tiles: f0,f1=q f2,f3=k
F32 = mybir.dt.float32
MM = mybir.dt.float32r
BF16 = mybir.dt.bfloat16

LAST_RESULTS = None
_CACHE = {}


def build(apply_qbias):
    nc = bacc.Bacc("TRN2", target_bir_lowering=False, debug=False,
                   num_devices=N_CORES)

    VW = CI + HPC if apply_qbias else CI   # v-proj width (+wstar features)

    xt_d = nc.dram_tensor("xt", [C, T], BF16, kind="ExternalInput").ap()
    wqk_d = nc.dram_tensor("wqk", [C, FQK, P], BF16, kind="ExternalInput").ap()
    wv_d = nc.dram_tensor("wv", [C, VW], BF16, kind="ExternalInput").ap()
    wp_d = nc.dram_tensor("wp", [CI, C], MM, kind="ExternalInput").ap()
    bp_d = nc.dram_tensor("bp", [P, CT], F32, kind="ExternalInput").ap()
    masks_d = nc.dram_tensor("masks", [P, 896], BF16, kind="ExternalInput").ap()
    vones_d = nc.dram_tensor("vones", [P, T1 * HPC], BF16, kind="ExternalInput").ap()
    onesc_d = nc.dram_tensor("onesc", [1, HD], MM, kind="ExternalInput").ap()
    out_d = nc.dram_tensor("out", [C, T], F32, kind="ExternalOutput").ap()
    out2_d = nc.dram_tensor("out2", [C, 512], F32, kind="ExternalOutput").ap()

    with tile.TileContext(nc) as tc:
        with (
            tc.tile_pool(name="const", bufs=1) as cp,
            tc.tile_pool(name="wk", bufs=2) as wk,
            tc.tile_pool(name="oqp", bufs=4) as oqp,
            tc.tile_pool(name="atp", bufs=16) as atp,
            tc.tile_pool(name="big", bufs=2, space="PSUM") as bigp,
            tc.tile_pool(name="ps2", bufs=2, space="PSUM") as ps2p,
            tc.tile_pool(name="pavp", bufs=2, space="PSUM") as pavp,
        ):
            # ---- resident SBUF tensors -------------------------------------
            xt_sb = cp.tile([P, CT, T], BF16)           # x^T            32 KB
            wqk_sb = cp.tile([P, CT, FQK, P], BF16)     # W_qk_eff^T      8 KB
            wv_sb = cp.tile([P, CT, VW], BF16)          # W_v_eff^T       4 KB
            wp_sb = cp.tile([P, 2, CT, P], MM)          # W_p_eff^T slice 8 KB
            bp_sb = cp.tile([P, CT], F32)
            qk_sb = cp.tile([P, FQK, T], BF16)          # q,k feature-major 16 KB
            v_sb = cp.tile([P, T1, HPC, HD + 1], BF16)  # v natural + ones 8.1 KB
            yt_sb = cp.tile([P, 2, T], MM)              # y^T (ci-major)  16 KB
            masks = cp.tile([P, 896], BF16)             # causal masks  1.75 KB
            ones_sb = cp.tile([1, HD], MM)              # PE-bcast stationary
            if apply_qbias:
                eqb_sb = cp.tile([P, T1, HPC, 1], F32)  # exp(0.125*bq.k)

            # ---- input DMAs ------------------------------------------------
            # j0 x-blocks + qk weights first, split in partition halves so the
            # first qk matmul can start ~11us in (queue time per DMA halves)
            for c in range(CT):
                for hh in range(2):
                    nc.sync.dma_start(
                        out=xt_sb[64 * hh:64 * (hh + 1), c, 0:512],
                        in_=xt_d[c * P + 64 * hh:c * P + 64 * (hh + 1), 0:512])
                    nc.sync.dma_start(
                        out=wqk_sb[64 * hh:64 * (hh + 1), c, :, :],
                        in_=wqk_d[c * P + 64 * hh:c * P + 64 * (hh + 1), :, :])
            for c in range(CT):
                nc.sync.dma_start(out=wv_sb[:, c, :], in_=wv_d[c * P:(c + 1) * P, :])
            nc.sync.dma_start(out=masks[:], in_=masks_d[:])
            nc.sync.dma_start(out=v_sb[:, :, :, HD:HD + 1], in_=vones_d[:])
            nc.sync.dma_start(out=ones_sb[:], in_=onesc_d[:])
            for j in range(1, T5):
                for c in range(CT):
                    nc.sync.dma_start(out=xt_sb[:, c, j * 512:(j + 1) * 512],
                                      in_=xt_d[c * P:(c + 1) * P,
                                               j * 512:(j + 1) * 512])
            for ci in range(2):
                nc.sync.dma_start(out=wp_sb[:, ci, :, :],
                                  in_=wp_d[ci * P:(ci + 1) * P, :])
            nc.sync.dma_start(out=bp_sb[:, :], in_=bp_d[:, :])

            # ---- emit helpers ----------------------------------------------
            def emit_qk(f, j):
                # qk^T f-tile: [128 feat, 512 t] = W_qk_eff^T @ x^T
                pq = bigp.tile([P, 512], F32, tag="big", name=f"pq{f}_{j}")
                for c in range(CT):
                    nc.tensor.matmul(pq[:], wqk_sb[:, c, f, :],
                                     xt_sb[:, c, j * 512:(j + 1) * 512],
                                     start=(c == 0), stop=(c == CT - 1))
                nc.vector.tensor_scalar_mul(
                    qk_sb[:, f, j * 512:(j + 1) * 512], pq[:], 1.0)

            # wstar features (variant B) interleave per head: [64 v | 1 star]
            HDV = HD + 1 if apply_qbias else HD

            def emit_v(i):
                # V natural: [128 t, VW feats] = x @ W_v_eff^T
                pv = bigp.tile([P, HPC, HDV], F32, tag="big", name=f"pv{i}")
                for c in range(CT):
                    nc.tensor.matmul(pv[:], xt_sb[:, c, i * P:(i + 1) * P],
                                     wv_sb[:, c, :],
                                     start=(c == 0), stop=(c == CT - 1))
                nc.scalar.copy(v_sb[:, i, :, 0:HD], pv[:, :, 0:HD])
                if apply_qbias:
                    nc.scalar.activation(eqb_sb[:, i, :, :],
                                         pv[:, :, HD:HD + 1],
                                         mybir.ActivationFunctionType.Exp,
                                         scale=0.125)

            class Unit:
                def __init__(self, h, j):
                    self.h, self.j, self.ni = h, j, 4 * j + 4
                    self.pav = None
                    self.ats = []

                def at(self, i):
                    return self.ats[i // 2][:, i % 2, :]

            def emit_scores(u, pair):
                # two k-tiles of S^T into one 2-bank PSUM tile + one Exp
                h, j = u.h, u.j
                pq_base = (h % 2) * HD
                fq, fk = h // 2, 2 + h // 2
                qt = qk_sb[pq_base:pq_base + HD, fq, j * 512:(j + 1) * 512]
                ps2t = ps2p.tile([P, 2, 512], F32, tag="ps2",
                                 name=f"ps{h}_{j}_{pair}")
                for d in range(2):
                    i = 2 * pair + d
                    kt = qk_sb[pq_base:pq_base + HD, fk, i * P:(i + 1) * P]
                    nc.tensor.matmul(ps2t[:, d, :], kt, qt, start=True, stop=True)
                at2 = atp.tile([P, 2, 512], BF16, tag="at",
                               name=f"at{h}_{j}_{pair}")
                nc.scalar.activation(at2[:, :, :], ps2t[:, :, :],
                                     mybir.ActivationFunctionType.Exp,
                                     scale=0.125)
                for d in range(2):
                    i = 2 * pair + d
                    a = i - 4 * j
                    if a >= 0:
                        eng = nc.gpsimd if a == 3 else nc.vector
                        eng.tensor_tensor(
                            at2[:, d, :], at2[:, d, :],
                            masks[:, 384 - 128 * a:896 - 128 * a],
                            mybir.AluOpType.mult)
                    if apply_qbias:
                        nc.vector.tensor_scalar_mul(
                            at2[:, d, :], at2[:, d, :], eqb_sb[:, i, u.h, :])
                u.ats.append(at2)

            def emit_av(u, i):
                if u.pav is None:
                    u.pav = pavp.tile([HD + 1, 512], F32, tag="pav",
                                      name=f"pav{u.h}_{u.j}")
                nc.tensor.matmul(u.pav[:], v_sb[:, i, u.h, :], u.at(i),
                                 start=(i == 0), stop=(i == u.ni - 1))

            def emit_bsb(u):
                # denominator row (64) of pav -> SBUF, off the critical path
                bsb = wk.tile([1, 512], MM, tag="bsb", name=f"bsb{u.h}_{u.j}")
                nc.vector.tensor_scalar_mul(bsb[:], u.pav[HD:HD + 1, :], 1.0)
                return (u, bsb)

            def emit_norm(u, bsb):
                # y^T = yu^T * (1/D); D broadcast to 64 partitions via PE
                h, j = u.h, u.j
                pb = bigp.tile([HD, 512], F32, tag="big", name=f"pb{h}_{j}")
                nc.tensor.matmul(pb[:], ones_sb[:], bsb[:], start=True, stop=True)
                rsb = wk.tile([HD, 512], F32, tag="rsb", name=f"rsb{h}_{j}")
                nc.vector.reciprocal_approx_fast(out=rsb[:], in_=pb[:])
                if h % 2 == 0:
                    nc.vector.tensor_tensor(
                        yt_sb[0:HD, h // 2, j * 512:(j + 1) * 512],
                        u.pav[0:HD, :], rsb[:], mybir.AluOpType.mult)
                else:
                    tsb = wk.tile([HD, 512], MM, tag="tsb", name=f"tsb{h}_{j}")
                    nc.vector.tensor_tensor(tsb[:], u.pav[0:HD, :], rsb[:],
                                            mybir.AluOpType.mult)
                    for half in range(2):
                        nc.sync.dma_start(
                            out=yt_sb[HD + 32 * half:HD + 32 * (half + 1),
                                      h // 2, j * 512:(j + 1) * 512],
                            in_=tsb[32 * half:32 * (half + 1), :])

            def emit_proj(j, co, ci=None):
                # ci=None: full (both halves); ci=0/1: partial for the j3
                # tail split (ci1 lands in out2, summed on host)
                po = bigp.tile([P, 512], F32, tag="big", name=f"po{j}_{co}_{ci}")
                cis = range(2) if ci is None else (ci,)
                for k, c_ in enumerate(cis):
                    nc.tensor.matmul(po[:], wp_sb[:, c_, co, :],
                                     yt_sb[:, c_, j * 512:(j + 1) * 512],
                                     start=(k == 0), stop=(k == len(cis) - 1))
                oq = oqp.tile([P, 512], F32, tag="oq", name=f"oq{j}_{co}_{ci}")
                bias = 0.0 if ci == 1 else bp_sb[:, co:co + 1]
                if co % 2 == 0:
                    if ci == 1:
                        nc.vector.tensor_scalar_mul(oq[:], po[:], 1.0)
                    else:
                        nc.vector.tensor_scalar_add(oq[:], po[:], bias)
                else:
                    nc.scalar.activation(oq[:], po[:],
                                         mybir.ActivationFunctionType.Identity,
                                         bias=bias)
                if ci == 1:
                    dst = out2_d[co * P:(co + 1) * P, :]
                else:
                    dst = out_d[co * P:(co + 1) * P, j * 512:(j + 1) * 512]
                for qt_ in range(4):
                    nc.sync.dma_start(
                        out=dst[32 * qt_:32 * (qt_ + 1), :],
                        in_=oq[32 * qt_:32 * (qt_ + 1), :])

            # ---- schedule: j-major attention groups, software pipelined ----
            # qk j0 + v 0-3 up front; attention group j0 starts right after.
            # Later qk j-tiles, v tiles, and the projection of each finished
            # j-group dribble into the attention stream as PE filler: keeps
            # the tensor engine p-state at max clock and spreads the output
            # DMA across the whole run instead of a tail burst.
            for f in (0, 2):
                emit_qk(f, 0)
            for i in range(4):
                emit_v(i)

            vq = list(range(4, T1))
            qkq = [(1, 0), (3, 0)] + \
                  [(f, j) for j in range(1, T5) for f in (0, 2, 1, 3)]
            projq = []
            fill_credit = [0.0]

            def pop_filler(credit):
                fill_credit[0] += credit
                while fill_credit[0] >= 1.0 and (vq or qkq or projq):
                    fill_credit[0] -= 1.0
                    if vq:
                        emit_v(vq.pop(0))
                    elif qkq:
                        f, j = qkq.pop(0)
                        emit_qk(f, j)
                    else:
                        emit_proj(*projq.pop(0))

            units = [Unit(h, j) for j in range(T5) for h in range(HPC)]
            state = {"prev": None, "pend": None}

            def section(u):
                prev = state["prev"]
                # force-pop fillers whose consumers are emitted in this
                # section (same-queue ordering would deadlock otherwise)
                if prev is not None:
                    while vq and vq[0] < prev.ni:
                        emit_v(vq.pop(0))
                if u is not None:
                    # scores of u need q-tile h//2 and k-tile 2+h//2 of block j
                    fneed = (u.h // 2, 2 + u.h // 2)
                    while qkq and (qkq[0][1] < u.j or
                                   (qkq[0][1] == u.j and
                                    any((f_, u.j) in qkq for f_ in fneed))):
                        f, j = qkq.pop(0)
                        emit_qk(f, j)
                npair = u.ni // 2 if u is not None else 0
                prev_avs = list(range(prev.ni)) if prev is not None else []
                # pace prev AVs to finish ~2 pairs early so the denominator
                # row copy (DVE) completes before the PE broadcast at the
                # section end
                avail = max(1, npair - 2)
                per_pair = -(-len(prev_avs) // avail) if prev_avs else 0
                for pair in range(npair):
                    emit_scores(u, pair)
                    for _ in range(per_pair):
                        if prev_avs:
                            emit_av(prev, prev_avs.pop(0))
                    if prev is not None and not prev_avs and state["pend"] is None:
                        state["pend"] = emit_bsb(prev)
                    pop_filler(1.0 if vq or qkq else 0.6)
                while prev_avs:
                    emit_av(prev, prev_avs.pop(0))
                if prev is not None and state["pend"] is None:
                    state["pend"] = emit_bsb(prev)
                if state["pend"] is not None:
                    emit_norm(*state["pend"])
                    state["pend"] = None
                if prev is not None and prev.h == HPC - 1 and prev.j < T5 - 1:
                    projq.extend((prev.j, co, None) for co in range(CT))
                if prev is not None and prev.h == 1 and prev.j == T5 - 1:
                    projq.extend((prev.j, co, 0) for co in range(CT))
                if prev is not None and prev.h == HPC - 1 and prev.j == T5 - 1:
                    projq.extend((prev.j, co, 1) for co in range(CT))
                state["prev"] = u

            for u in units:
                section(u)
            section(None)     # drain last unit
            while vq or qkq or projq:
                pop_filler(1.0)

    nc.compile()
    return nc


def _shard_inputs(x, w_attn, b_attn, lora_a_attn, lora_b_attn, w_proj, b_proj,
                  lora_a_proj, lora_b_proj, apply_qbias):
    f32 = np.float32
    import ml_dtypes
    bf16 = ml_dtypes.bfloat16

    x = np.asarray(x, f32)
    w_attn = np.asarray(w_attn, f32)
    b_attn = np.asarray(b_attn, f32)
    w_proj = np.asarray(w_proj, f32)
    b_proj = np.asarray(b_proj, f32)

    # exact host folds: LoRA into weights
    wa_eff = w_attn + LORA_SCALE * (
        np.asarray(lora_b_attn, f32) @ np.asarray(lora_a_attn, f32))
    wp_eff = w_proj + LORA_SCALE * (
        np.asarray(lora_b_proj, f32) @ np.asarray(lora_a_proj, f32))

    # masks[p, z] = 1.0 if z >= p + 384 else 0.0
    pp, zz = np.meshgrid(np.arange(P), np.arange(896), indexing="ij")
    masks = (zz >= pp + 384).astype(bf16)
    vones = np.ones((P, T1 * HPC), bf16)
    onesc = np.ones((1, HD), f32)
    in_maps = []
    for core in range(N_CORES):
        b = core // GPB
        heads = [(core % GPB) * HPC + k for k in range(HPC)]
        q_idx = np.concatenate([np.arange(h * HD, (h + 1) * HD) for h in heads])
        k_idx = q_idx + C
        v_idx = q_idx + 2 * C
        qk_idx = np.concatenate([q_idx, k_idx])
        wqk_t = np.ascontiguousarray(
            wa_eff[qk_idx].T.reshape(C, FQK, P))           # (C, 4, 128)
        wv_t = wa_eff[v_idx].T                             # (C, 256)
        if apply_qbias:
            # wstar[:, h] = W_k_eff(head h)^T @ b_q(head h); interleave so the
            # v-phase emits [64 v cols | 1 wstar col] per head
            wstar = np.stack(
                [wa_eff[C + h * HD:C + (h + 1) * HD].T
                 @ b_attn[h * HD:(h + 1) * HD] for h in heads], axis=1)
            wv_t = np.concatenate(
                [wv_t.reshape(C, HPC, HD), wstar[:, :, None]],
                axis=2).reshape(C, HPC * (HD + 1))         # (C, 260)
        wp_t = np.ascontiguousarray(wp_eff[:, q_idx].T)    # (256, C)
        # v-bias folds into the projection bias (softmax weights sum to 1)
        bp = wp_t.T @ b_attn[v_idx]
        if core % GPB == 0:
            bp = bp + b_proj
        in_maps.append({
            "xt": np.ascontiguousarray(x[b].T).astype(bf16),
            "wqk": wqk_t.astype(bf16),
            "wv": np.ascontiguousarray(wv_t).astype(bf16),
            "wp": wp_t,
            "bp": np.ascontiguousarray(bp.reshape(CT, P).T),
            "masks": masks, "vones": vones, "onesc": onesc,
        })
    return in_maps


def kernel(x, w_attn, b_attn, lora_a_attn, lora_b_attn, w_proj, b_proj,
           lora_a_proj, lora_b_proj, n_head):
    global LAST_RESULTS
    assert int(n_head) == H
    apply_qbias = bool(np.any(np.asarray(b_attn)[:C] != 0))
    key = ("nc", apply_qbias)
    if key not in _CACHE:
        _CACHE[key] = build(apply_qbias)
    nc = _CACHE[key]
    in_maps = _shard_inputs(x, w_attn, b_attn, lora_a_attn, lora_b_attn,
                            w_proj, b_proj, lora_a_proj, lora_b_proj,
                            apply_qbias)
    res = run_bass_kernel_spmd(
        nc, in_maps, core_ids=list(range(N_CORES)),
        trace=bool(os.environ.get("BASS_KERNEL_TRACE")))
    LAST_RESULTS = res
    out = np.zeros((B, C, T), np.float32)
    for core in range(N_CORES):
        out[core // GPB] += res.results[core]["out"]
        out[core // GPB][:, (T5 - 1) * 512:] += res.results[core]["out2"]
    return np.ascontiguousarray(out.transpose(0, 2, 1))


# revision 18
# speedup vs baseline: 1.0974x; 1.0974x over previous
"""Trainium2 Bass kernel: causal multi-head attention with LoRA (B=2, T=2048,
C=1024, 16 heads, r=16), SPMD across 8 NeuronCores.

Sharding: core = (batch, head-group-of-4). QKV + attention are fully local per
core; the output projection is a partial sum over each core's 256 y-features,
reduced on host.

Host-side exact folds (no HW cost):
  - LoRA:  W_eff = W + (1/r) * B @ A        (both attn and proj)
  - k-bias: drops out of softmax (constant shift per query)
  - v-bias: y = sum(p*(v+bv)) = sum(p*v) + bv  ->  folded into proj bias
  - q-bias: adds (bq . k_t) to every score column; k is linear in x, so it is
    one extra projection feature (wstar = W_k_eff^T bq); applied post-exp as a
    per-partition multiply only when any q-bias is nonzero (variant flag).

Device schedule (single NeuronCore, emission order == per-engine order):
  qk f0/f2 proj -> v tiles 0-3 -> attention units (h-major), software
  pipelined: unit U's score matmuls interleave with unit U-1's AV matmuls so
  the PE never waits on the exp chain; remaining v tiles and qk f1/f3 are
  dribbled into the attention stream as PE filler to keep the tensor engine
  p-state at max clock. Scores are written as [128,2,512] 2-bank PSUM tiles so
  one Exp instruction covers two k-tiles (halves Act instruction count).
  Output projection drains at the tail, alternating Act/DVE PSUM reads.
"""
import os
import sys

sys.path.insert(0, "/opt/trn_rl_repo")

import numpy as np

import concourse.bass as bass  # noqa: F401
import concourse.bacc as bacc
import concourse.tile as tile
import concourse.mybir as mybir
from concourse.bass_utils import run_bass_kernel_spmd

B, T, C = 2, 2048, 1024
H, HD = 16, 64
R = 16
LORA_SCALE = 1.0 / R
N_CORES = 8
GPB = N_CORES // B          # core groups per batch = 4
HPC = H // GPB              # heads per core = 4
CI = HPC * HD               # per-core y features = 256
P = 128
T5 = T // 512               # 4  (512-wide t tiles)
T1 = T // P                 # 16 (128-wide t tiles)
CT = C // P                 # 8  (128-wide c tiles)
FQK = 4                     # 128-wide qk feature tiles: f0,f1=q f2,f3=k
F32 = mybir.dt.float32
MM = mybir.dt.float32r
BF16 = mybir.dt.bfloat16

LAST_RESULTS = None
_CACHE = {}


def build(apply_qbias):
    nc = bacc.Bacc("TRN2", target_bir_lowering=False, debug=False,
                   num_devices=N_CORES)

    VW = CI + HPC if apply_qbias else CI   # v-proj width (+wstar features)

    xt_d = nc.dram_tensor("xt", [C, T], BF16, kind="ExternalInput").ap()
    wqk_d = nc.dram_tensor("wqk", [C, FQK, P], BF16, kind="ExternalInput").ap()
    wv_d = nc.dram_tensor("wv", [C, VW], BF16, kind="ExternalInput").ap()
    wp_d = nc.dram_tensor("wp", [CI, C], MM, kind="ExternalInput").ap()
    bp_d = nc.dram_tensor("bp", [P, CT], F32, kind="ExternalInput").ap()
    masks_d = nc.dram_tensor("masks", [P, 896], BF16, kind="ExternalInput").ap()
    vones_d = nc.dram_tensor("vones", [P, T1 * HPC], BF16, kind="ExternalInput").ap()
    onesc_d = nc.dram_tensor("onesc", [1, HD], MM, kind="ExternalInput").ap()
    out_d = nc.dram_tensor("out", [C, T], F32, kind="ExternalOutput").ap()

    with tile.TileContext(nc) as tc:
        with (
            tc.tile_pool(name="const", bufs=1) as cp,
            tc.tile_pool(name="wk", bufs=2) as wk,
            tc.tile_pool(name="oqp", bufs=4) as oqp,
            tc.tile_pool(name="atp", bufs=16) as atp,
            tc.tile_pool(name="big", bufs=2, space="PSUM") as bigp,
            tc.tile_pool(name="ps2", bufs=2, space="PSUM") as ps2p,
            tc.tile_pool(name="pavp", bufs=2, space="PSUM") as pavp,
        ):
            # ---- resident SBUF tensors -------------------------------------
            xt_sb = cp.tile([P, CT, T], BF16)           # x^T            32 KB
            wqk_sb = cp.tile([P, CT, FQK, P], BF16)     # W_qk_eff^T      8 KB
            wv_sb = cp.tile([P, CT, VW], BF16)          # W_v_eff^T       4 KB
            wp_sb = cp.tile([P, 2, CT, P], MM)          # W_p_eff^T slice 8 KB
            bp_sb = cp.tile([P, CT], F32)
            qk_sb = cp.tile([P, FQK, T], BF16)          # q,k feature-major 16 KB
            v_sb = cp.tile([P, T1, HPC, HD + 1], BF16)  # v natural + ones 8.1 KB
            yt_sb = cp.tile([P, 2, T], MM)              # y^T (ci-major)  16 KB
            masks = cp.tile([P, 896], BF16)             # causal masks  1.75 KB
            ones_sb = cp.tile([1, HD], MM)              # PE-bcast stationary
            if apply_qbias:
                eqb_sb = cp.tile([P, T1, HPC, 1], F32)  # exp(0.125*bq.k)

            # ---- input DMAs ------------------------------------------------
            # j0 x-blocks + qk weights first, split in partition halves so the
            # first qk matmul can start ~11us in (queue time per DMA halves)
            for c in range(CT):
                for hh in range(2):
                    nc.sync.dma_start(
                        out=xt_sb[64 * hh:64 * (hh + 1), c, 0:512],
                        in_=xt_d[c * P + 64 * hh:c * P + 64 * (hh + 1), 0:512])
                    nc.sync.dma_start(
                        out=wqk_sb[64 * hh:64 * (hh + 1), c, :, :],
                        in_=wqk_d[c * P + 64 * hh:c * P + 64 * (hh + 1), :, :])
            for c in range(CT):
                nc.sync.dma_start(out=wv_sb[:, c, :], in_=wv_d[c * P:(c + 1) * P, :])
            nc.sync.dma_start(out=masks[:], in_=masks_d[:])
            nc.sync.dma_start(out=v_sb[:, :, :, HD:HD + 1], in_=vones_d[:])
            nc.sync.dma_start(out=ones_sb[:], in_=onesc_d[:])
            for j in range(1, T5):
                for c in range(CT):
                    nc.sync.dma_start(out=xt_sb[:, c, j * 512:(j + 1) * 512],
                                      in_=xt_d[c * P:(c + 1) * P,
                                               j * 512:(j + 1) * 512])
            for ci in range(2):
                nc.sync.dma_start(out=wp_sb[:, ci, :, :],
                                  in_=wp_d[ci * P:(ci + 1) * P, :])
            nc.sync.dma_start(out=bp_sb[:, :], in_=bp_d[:, :])

            # ---- emit helpers ----------------------------------------------
            def emit_qk(f, j):
                # qk^T f-tile: [128 feat, 512 t] = W_qk_eff^T @ x^T
                pq = bigp.tile([P, 512], F32, tag="big", name=f"pq{f}_{j}")
                for c in range(CT):
                    nc.tensor.matmul(pq[:], wqk_sb[:, c, f, :],
                                     xt_sb[:, c, j * 512:(j + 1) * 512],
                                     start=(c == 0), stop=(c == CT - 1))
                nc.vector.tensor_scalar_mul(
                    qk_sb[:, f, j * 512:(j + 1) * 512], pq[:], 1.0)

            # wstar features (variant B) interleave per head: [64 v | 1 star]
            HDV = HD + 1 if apply_qbias else HD

            def emit_v(i):
                # V natural: [128 t, VW feats] = x @ W_v_eff^T
                pv = bigp.tile([P, HPC, HDV], F32, tag="big", name=f"pv{i}")
                for c in range(CT):
                    nc.tensor.matmul(pv[:], xt_sb[:, c, i * P:(i + 1) * P],
                                     wv_sb[:, c, :],
                                     start=(c == 0), stop=(c == CT - 1))
                nc.scalar.copy(v_sb[:, i, :, 0:HD], pv[:, :, 0:HD])
                if apply_qbias:
                    nc.scalar.activation(eqb_sb[:, i, :, :],
                                         pv[:, :, HD:HD + 1],
                                         mybir.ActivationFunctionType.Exp,
                                         scale=0.125)

            class Unit:
                def __init__(self, h, j):
                    self.h, self.j, self.ni = h, j, 4 * j + 4
                    self.pav = None
                    self.ats = []

                def at(self, i):
                    return self.ats[i // 2][:, i % 2, :]

            def emit_scores(u, pair):
                # two k-tiles of S^T into one 2-bank PSUM tile + one Exp
                h, j = u.h, u.j
                pq_base = (h % 2) * HD
                fq, fk = h // 2, 2 + h // 2
                qt = qk_sb[pq_base:pq_base + HD, fq, j * 512:(j + 1) * 512]
                ps2t = ps2p.tile([P, 2, 512], F32, tag="ps2",
                                 name=f"ps{h}_{j}_{pair}")
                for d in range(2):
                    i = 2 * pair + d
                    kt = qk_sb[pq_base:pq_base + HD, fk, i * P:(i + 1) * P]
                    nc.tensor.matmul(ps2t[:, d, :], kt, qt, start=True, stop=True)
                at2 = atp.tile([P, 2, 512], BF16, tag="at",
                               name=f"at{h}_{j}_{pair}")
                nc.scalar.activation(at2[:, :, :], ps2t[:, :, :],
                                     mybir.ActivationFunctionType.Exp,
                                     scale=0.125)
                for d in range(2):
                    i = 2 * pair + d
                    a = i - 4 * j
                    if a >= 0:
                        last = (u.h == HPC - 1 and u.j == T5 - 1)
                        eng = nc.gpsimd if (a == 3 and not last) else nc.vector
                        eng.tensor_tensor(
                            at2[:, d, :], at2[:, d, :],
                            masks[:, 384 - 128 * a:896 - 128 * a],
                            mybir.AluOpType.mult)
                    if apply_qbias:
                        nc.vector.tensor_scalar_mul(
                            at2[:, d, :], at2[:, d, :], eqb_sb[:, i, u.h, :])
                u.ats.append(at2)

            def emit_av(u, i):
                if u.pav is None:
                    u.pav = pavp.tile([HD + 1, 512], F32, tag="pav",
                                      name=f"pav{u.h}_{u.j}")
                nc.tensor.matmul(u.pav[:], v_sb[:, i, u.h, :], u.at(i),
                                 start=(i == 0), stop=(i == u.ni - 1))

            def emit_bsb(u):
                # denominator row (64) of pav -> SBUF, off the critical path
                bsb = wk.tile([1, 512], MM, tag="bsb", name=f"bsb{u.h}_{u.j}")
                nc.vector.tensor_scalar_mul(bsb[:], u.pav[HD:HD + 1, :], 1.0)
                return (u, bsb)

            def emit_norm(u, bsb):
                # y^T = yu^T * (1/D); D broadcast to 64 partitions via PE
                h, j = u.h, u.j
                pb = bigp.tile([HD, 512], F32, tag="big", name=f"pb{h}_{j}")
                nc.tensor.matmul(pb[:], ones_sb[:], bsb[:], start=True, stop=True)
                rsb = wk.tile([HD, 512], F32, tag="rsb", name=f"rsb{h}_{j}")
                nc.vector.reciprocal_approx_fast(out=rsb[:], in_=pb[:])
                if h % 2 == 0:
                    nc.vector.tensor_tensor(
                        yt_sb[0:HD, h // 2, j * 512:(j + 1) * 512],
                        u.pav[0:HD, :], rsb[:], mybir.AluOpType.mult)
                else:
                    tsb = wk.tile([HD, 512], MM, tag="tsb", name=f"tsb{h}_{j}")
                    nc.vector.tensor_tensor(tsb[:], u.pav[0:HD, :], rsb[:],
                                            mybir.AluOpType.mult)
                    for half in range(2):
                        nc.sync.dma_start(
                            out=yt_sb[HD + 32 * half:HD + 32 * (half + 1),
                                      h // 2, j * 512:(j + 1) * 512],
                            in_=tsb[32 * half:32 * (half + 1), :])

            def emit_proj(j, co):
                po = bigp.tile([P, 512], F32, tag="big", name=f"po{j}_{co}")
                for ci in range(2):
                    nc.tensor.matmul(po[:], wp_sb[:, ci, co, :],
                                     yt_sb[:, ci, j * 512:(j + 1) * 512],
                                     start=(ci == 0), stop=(ci == 1))
                oq = oqp.tile([P, 512], F32, tag="oq", name=f"oq{j}_{co}")
                if co % 2 == 0:
                    nc.vector.tensor_scalar_add(oq[:], po[:], bp_sb[:, co:co + 1])
                else:
                    nc.scalar.activation(oq[:], po[:],
                                         mybir.ActivationFunctionType.Identity,
                                         bias=bp_sb[:, co:co + 1])
                dst = out_d[co * P:(co + 1) * P, j * 512:(j + 1) * 512]
                for qt_ in range(4):
                    nc.sync.dma_start(
                        out=dst[32 * qt_:32 * (qt_ + 1), :],
                        in_=oq[32 * qt_:32 * (qt_ + 1), :])

            # ---- schedule: j-major attention groups, software pipelined ----
            # qk j0 + v 0-3 up front; attention group j0 starts right after.
            # Later qk j-tiles, v tiles, and the projection of each finished
            # j-group dribble into the attention stream as PE filler: keeps
            # the tensor engine p-state at max clock and spreads the output
            # DMA across the whole run instead of a tail burst.
            # prelude: qk f0/f2 (j0) and v 0-3 interleaved per c-tile over
            # six concurrent PSUM accumulators, so the PE tracks the x DMA
            # arrival pipeline instead of stalling per phase
            pq0 = bigp.tile([P, 512], F32, tag="big", name="pq0")
            pq2 = bigp.tile([P, 512], F32, tag="big", name="pq2")
            pvt = []
            for i in range(4):
                pool, tg = ((ps2p, "ps2") if i < 2 else (pavp, "pav"))
                pvt.append(pool.tile([P, HPC, HDV], F32, tag=tg, name=f"pvp{i}"))
            for c in range(CT):
                st, sp = (c == 0), (c == CT - 1)
                nc.tensor.matmul(pq0[:], wqk_sb[:, c, 0, :],
                                 xt_sb[:, c, 0:512], start=st, stop=sp)
                nc.tensor.matmul(pq2[:], wqk_sb[:, c, 2, :],
                                 xt_sb[:, c, 0:512], start=st, stop=sp)
                for i in range(4):
                    nc.tensor.matmul(pvt[i][:], xt_sb[:, c, i * P:(i + 1) * P],
                                     wv_sb[:, c, :], start=st, stop=sp)
            nc.vector.tensor_scalar_mul(qk_sb[:, 0, 0:512], pq0[:], 1.0)
            nc.vector.tensor_scalar_mul(qk_sb[:, 2, 0:512], pq2[:], 1.0)
            for i in range(4):
                nc.scalar.copy(v_sb[:, i, :, 0:HD], pvt[i][:, :, 0:HD])
                if apply_qbias:
                    nc.scalar.activation(eqb_sb[:, i, :, :],
                                         pvt[i][:, :, HD:HD + 1],
                                         mybir.ActivationFunctionType.Exp,
                                         scale=0.125)

            vq = list(range(4, T1))
            qkq = [(1, 0), (3, 0)] + \
                  [(f, j) for j in range(1, T5) for f in (0, 2, 1, 3)]
            projq = []
            fill_credit = [0.0]

            toggle = [0]

            def pop_filler(credit):
                fill_credit[0] += credit
                while fill_credit[0] >= 1.0 and (vq or qkq or projq):
                    fill_credit[0] -= 1.0
                    toggle[0] ^= 1
                    if vq and (toggle[0] or not qkq):
                        emit_v(vq.pop(0))
                    elif qkq:
                        f, j = qkq.pop(0)
                        emit_qk(f, j)
                    elif vq:
                        emit_v(vq.pop(0))
                    else:
                        emit_proj(*projq.pop(0))

            units = [Unit(h, j) for j in range(T5) for h in range(HPC)]
            state = {"prev": None, "pend": None}

            def section(u):
                prev = state["prev"]
                # force-pop fillers whose consumers are emitted in this
                # section (same-queue ordering would deadlock otherwise)
                if prev is not None:
                    while vq and vq[0] < prev.ni:
                        emit_v(vq.pop(0))
                if u is not None:
                    # scores of u need q-tile h//2 and k-tile 2+h//2 of block j
                    fneed = (u.h // 2, 2 + u.h // 2)
                    while qkq and (qkq[0][1] < u.j or
                                   (qkq[0][1] == u.j and
                                    any((f_, u.j) in qkq for f_ in fneed))):
                        f, j = qkq.pop(0)
                        emit_qk(f, j)
                npair = u.ni // 2 if u is not None else 0
                prev_avs = list(range(prev.ni)) if prev is not None else []
                # pace prev AVs to finish ~2 pairs early so the denominator
                # row copy (DVE) completes before the PE broadcast at the
                # section end
                avail = max(1, npair - 2)
                per_pair = -(-len(prev_avs) // avail) if prev_avs else 0
                for pair in range(npair):
                    emit_scores(u, pair)
                    for _ in range(per_pair):
                        if prev_avs:
                            emit_av(prev, prev_avs.pop(0))
                    if prev is not None and not prev_avs and state["pend"] is None:
                        state["pend"] = emit_bsb(prev)
                    pop_filler(1.2 if vq or qkq else 0.6)
                while prev_avs:
                    emit_av(prev, prev_avs.pop(0))
                if prev is not None and state["pend"] is None:
                    state["pend"] = emit_bsb(prev)
                if state["pend"] is not None:
                    emit_norm(*state["pend"])
                    state["pend"] = None
                if prev is not None and prev.h == HPC - 1:
                    projq.extend((prev.j, co) for co in range(CT))
                state["prev"] = u

            for u in units:
                section(u)
            section(None)     # drain last unit
            while vq or qkq or projq:
                pop_filler(1.0)

    nc.compile()
    return nc


def _shard_inputs(x, w_attn, b_attn, lora_a_attn, lora_b_attn, w_proj, b_proj,
                  lora_a_proj, lora_b_proj, apply_qbias):
    f32 = np.float32
    import ml_dtypes
    bf16 = ml_dtypes.bfloat16

    x = np.asarray(x, f32)
    w_attn = np.asarray(w_attn, f32)
    b_attn = np.asarray(b_attn, f32)
    w_proj = np.asarray(w_proj, f32)
    b_proj = np.asarray(b_proj, f32)

    # exact host folds: LoRA into weights
    wa_eff = w_attn + LORA_SCALE * (
        np.asarray(lora_b_attn, f32) @ np.asarray(lora_a_attn, f32))
    wp_eff = w_proj + LORA_SCALE * (
        np.asarray(lora_b_proj, f32) @ np.asarray(lora_a_proj, f32))

    # masks[p, z] = 1.0 if z >= p + 384 else 0.0
    pp, zz = np.meshgrid(np.arange(P), np.arange(896), indexing="ij")
    masks = (zz >= pp + 384).astype(bf16)
    vones = np.ones((P, T1 * HPC), bf16)
    onesc = np.ones((1, HD), f32)
    in_maps = []
    for core in range(N_CORES):
        b = core // GPB
        heads = [(core % GPB) * HPC + k for k in range(HPC)]
        q_idx = np.concatenate([np.arange(h * HD, (h + 1) * HD) for h in heads])
        k_idx = q_idx + C
        v_idx = q_idx + 2 * C
        qk_idx = np.concatenate([q_idx, k_idx])
        wqk_t = np.ascontiguousarray(
            wa_eff[qk_idx].T.reshape(C, FQK, P))           # (C, 4, 128)
        wv_t = wa_eff[v_idx].T                             # (C, 256)
        if apply_qbias:
            # wstar[:, h] = W_k_eff(head h)^T @ b_q(head h); interleave so the
            # v-phase emits [64 v cols | 1 wstar col] per head
            wstar = np.stack(
                [wa_eff[C + h * HD:C + (h + 1) * HD].T
                 @ b_attn[h * HD:(h + 1) * HD] for h in heads], axis=1)
            wv_t = np.concatenate(
                [wv_t.reshape(C, HPC, HD), wstar[:, :, None]],
                axis=2).reshape(C, HPC * (HD + 1))         # (C, 260)
        wp_t = np.ascontiguousarray(wp_eff[:, q_idx].T)    # (256, C)
        # v-bias folds into the projection bias (softmax weights sum to 1)
        bp = wp_t.T @ b_attn[v_idx]
        if core % GPB == 0:
            bp = bp + b_proj
        in_maps.append({
            "xt": np.ascontiguousarray(x[b].T).astype(bf16),
            "wqk": wqk_t.astype(bf16),
            "wv": np.ascontiguousarray(wv_t).astype(bf16),
            "wp": wp_t,
            "bp": np.ascontiguousarray(bp.reshape(CT, P).T),
            "masks": masks, "vones": vones, "onesc": onesc,
        })
    return in_maps


def kernel(x, w_attn, b_attn, lora_a_attn, lora_b_attn, w_proj, b_proj,
           lora_a_proj, lora_b_proj, n_head):
    global LAST_RESULTS
    assert int(n_head) == H
    apply_qbias = bool(np.any(np.asarray(b_attn)[:C] != 0))
    key = ("nc", apply_qbias)
    if key not in _CACHE:
        _CACHE[key] = build(apply_qbias)
    nc = _CACHE[key]
    in_maps = _shard_inputs(x, w_attn, b_attn, lora_a_attn, lora_b_attn,
                            w_proj, b_proj, lora_a_proj, lora_b_proj,
                            apply_qbias)
    res = run_bass_kernel_spmd(
        nc, in_maps, core_ids=list(range(N_CORES)),
        trace=bool(os.environ.get("BASS_KERNEL_TRACE")))
    LAST_RESULTS = res
    out = np.zeros((B, C, T), np.float32)
    for core in range(N_CORES):
        out[core // GPB] += res.results[core]["out"]
    return np.ascontiguousarray(out.transpose(0, 2, 1))
